# revision 28
# baseline (speedup 1.0000x reference)
"""Block-diagonal MLP kernel for TRN2, 8 NeuronCores.

Computes out = x @ tanh(blocks * mask) where blocks is 4096x4096 with 16
diagonal 256x256 blocks (mask is the fixed block-diagonal pattern, all-ones
on the diagonal blocks). Off-diagonal entries of tanh(blocks*mask) are
tanh(0)=0 and contribute nothing, so only the 16 diagonal blocks matter:

    out[:, 256k:256(k+1)] = x[:, 256k:256(k+1)] @ tanh(B_k)

Sharding: block-parallel. Core c owns blocks 2c and 2c+1 (512 contiguous
k/n-columns) and streams all 8192 rows of x. Per-core device work:

    outT_shard[n, m] = sum_k b[k, n] * xT_shard[k, m]      (n, k local to core)

i.e. matmul(psum, lhsT=b_chunk[k,n], rhs=xT_chunk[k,m]) with the weight
chunk stationary. x is transposed on the host (layout prep, not compute) so
the contraction index k lands on SBUF partitions; the output comes back
transposed and is transposed back on the host during the gather.

Matmuls run as float32r (full-rate 1 cycle/row at N>=512, vs 4 cycles/row
for strict fp32) on f32 data via bitcast; accumulation is fp32 in PSUM.
"""

import numpy as np

import concourse.bass as bass
import concourse.mybir as mybir
import concourse.tile as tile
from concourse import bacc
from concourse.bass_utils import run_bass_kernel_spmd

N_CORES = 8
N_ROWS = 8192            # rows of x / out
D = 4096                 # layer size
BLOCK = 256              # block size
BLOCKS_PER_CORE = 2      # 16 blocks / 8 cores
K_PER_CORE = BLOCKS_PER_CORE * BLOCK   # 512 k (and n) columns per core
M_GROUP = 2048           # m columns per SBUF tile / DMA transfer (1 MiB)
N_GROUPS = N_ROWS // M_GROUP           # 4
MM_FREE = 512            # matmul moving free dim (one fp32 PSUM bank)
MT_PER_GROUP = M_GROUP // MM_FREE      # 4

USE_F32R = True

_nc_cache = None


def _build_nc():
    f32 = mybir.dt.float32
    mm_dt = mybir.dt.float32r if USE_F32R else f32

    # Bacc (not Bass): its compile() runs move_matmul_waits_to_ldweights and
    # generate_event_semaphores, which split multi-sem waits down to the 1
    # sync-wait-per-instruction the hardware supports.
    nc = bacc.Bacc("TRN2", enable_partition_id=False)
    xT = nc.dram_tensor("xT", [K_PER_CORE, N_ROWS], f32, kind="ExternalInput")
    bblk = nc.dram_tensor(
        "bblk", [BLOCKS_PER_CORE, BLOCK, BLOCK], f32, kind="ExternalInput"
    )
    outT = nc.dram_tensor("outT", [K_PER_CORE, N_ROWS], f32, kind="ExternalOutput")

    with tile.TileContext(nc) as tc:
        with (
            tc.tile_pool(name="bpool", bufs=1) as bpool,
            tc.tile_pool(name="xpool", bufs=4) as xpool,
            tc.tile_pool(name="xrpool", bufs=6) as xrpool,
            tc.tile_pool(name="opool", bufs=3) as opool,
            tc.tile_pool(name="pspool", bufs=8, space="PSUM") as pspool,
        ):
            # --- weights: load the 2 diagonal blocks, tanh once ---
            # column layout of b tiles: chunk (blk, kc) covers 256 cols at
            # (blk*2+kc)*256, holding b[k_chunk, n] for n in [0, 256).
            b_raw = bpool.tile([128, 1024], f32, name="b_raw")
            b_tanh = bpool.tile([128, 1024], f32, name="b_tanh")
            b_mm = bpool.tile([128, 1024], mm_dt, name="b_mm")
            # single DMA for all 4 [128, 256] weight chunks (keeps the tanh's
            # wait count at one semaphore): SBUF col chunk (blk*2+kc)*256
            # holds bblk[blk, kc*128 + p, n]
            nc.sync.dma_start(
                out=b_raw[:].rearrange("p (b kc n) -> p b kc n", b=2, kc=2),
                in_=bblk[:].rearrange("b (kc p) n -> p b kc n", p=128),
            )
            nc.scalar.activation(
                b_tanh[:], b_raw[:], mybir.ActivationFunctionType.Tanh
            )
            if USE_F32R:
                # fp32r is a distinct encoding: matmul operands must be
                # produced by an op that rounds to fp32r
                nc.vector.tensor_copy(b_mm[:], b_tanh[:])
            else:
                b_mm = b_tanh

            # --- stream xT in 1 MiB tiles: (q = k-chunk of 128, g = m group).
            # HWDGE loads into f32 landing tiles, DVE rounds to f32r.
            # Emission is interleaved with the compute loop: the Tile
            # scheduler's priority follows program order, so emitting all
            # casts up front lets them starve the PSUM evacuations on DVE at
            # group boundaries (measured as a multi-us store stall). Each
            # group's compute body emits the NEXT group's loads and casts so
            # evacs and casts alternate naturally on DVE.
            xts = {}

            def emit_load(q, g):
                t0 = xpool.tile([128, M_GROUP], f32, name=f"xl{q}_{g}", tag="xl")
                nc.sync.dma_start(
                    out=t0[:],
                    in_=xT[
                        q * 128 : (q + 1) * 128,
                        g * M_GROUP : (g + 1) * M_GROUP,
                    ],
                )
                return t0

            def emit_cast(q, g, t0):
                t = xrpool.tile([128, M_GROUP], mm_dt, name=f"xt{q}_{g}", tag="xt")
                nc.vector.tensor_copy(t[:], t0[:])
                xts[(q, g)] = t

            landing = {}
            for g in range(N_GROUPS):
                for q in range(4):
                    t0 = emit_load(q, g)
                    emit_cast(q, g, t0)

            # --- matmuls: psum[n 128, m 512] += b[k,n].T @ xT[k,m] over kc ---
            for g in range(N_GROUPS):
                sub = 0  # (blk, ncol) sub-iteration within the group
                for blk in range(BLOCKS_PER_CORE):
                    for ncol in range(2):  # n chunk of 128 within the block
                        out_sb = opool.tile([128, M_GROUP], f32, name="out_sb")
                        for mt in range(MT_PER_GROUP):
                            ps = pspool.tile([128, MM_FREE], f32, name="ps")
                            for kc in range(2):
                                q = blk * 2 + kc
                                lcol = ((blk * 2 + kc) * 2 + ncol) * 128
                                nc.tensor.matmul(
                                    ps[:],
                                    lhsT=b_mm[:, lcol : lcol + 128],
                                    rhs=xts[(q, g)][
                                        :, mt * MM_FREE : (mt + 1) * MM_FREE
                                    ],
                                    start=(kc == 0),
                                    stop=(kc == 1),
                                )
                            nc.vector.tensor_copy(
                                out_sb[:, mt * MM_FREE : (mt + 1) * MM_FREE], ps[:]
                            )
                        r0 = blk * 256 + ncol * 128
                        # stores go out on the ACT HWDGE ring so they don't
                        # queue behind the SP-ring loads
                        nc.scalar.dma_start(
                            out=outT[r0 : r0 + 128, g * M_GROUP : (g + 1) * M_GROUP],
                            in_=out_sb[:],
                        )
                        sub += 1
    nc.compile()
    return nc


def _get_nc():
    global _nc_cache
    if _nc_cache is None:
        _nc_cache = _build_nc()
    return _nc_cache


def _make_in_maps(x, blocks):
    xT = np.ascontiguousarray(x.T)  # [4096, 8192]
    in_maps = []
    for c in range(N_CORES):
        k0 = c * K_PER_CORE
        bstack = np.stack(
            [
                blocks[
                    k0 + i * BLOCK : k0 + (i + 1) * BLOCK,
                    k0 + i * BLOCK : k0 + (i + 1) * BLOCK,
                ]
                for i in range(BLOCKS_PER_CORE)
            ]
        )
        in_maps.append(
            {"xT": xT[k0 : k0 + K_PER_CORE, :], "bblk": np.ascontiguousarray(bstack)}
        )
    return in_maps


def _run(x, blocks, **spmd_kwargs):
    res = run_bass_kernel_spmd(
        _get_nc(), _make_in_maps(x, blocks), core_ids=list(range(N_CORES)),
        **spmd_kwargs,
    )
    out = np.empty((N_ROWS, D), np.float32)
    for c in range(N_CORES):
        out[:, c * K_PER_CORE : (c + 1) * K_PER_CORE] = res.results[c]["outT"].T
    return out, res


def kernel(x, blocks, mask=None):
    out, _ = _run(np.asarray(x), np.asarray(blocks))
    return out


# revision 30
# speedup vs baseline: 1.1778x; 1.1778x over previous
"""Block-diagonal MLP kernel for TRN2, 8 NeuronCores.

Computes out = x @ tanh(blocks * mask) where blocks is 4096x4096 with 16
diagonal 256x256 blocks (mask is the fixed block-diagonal pattern, all-ones
on the diagonal blocks). Off-diagonal entries of tanh(blocks*mask) are
tanh(0)=0 and contribute nothing, so only the 16 diagonal blocks matter:

    out[:, 256k:256(k+1)] = x[:, 256k:256(k+1)] @ tanh(B_k)

Sharding: block-parallel. Core c owns blocks 2c and 2c+1 (512 contiguous
k/n-columns) and streams all 8192 rows of x. Per-core device work:

    outT_shard[n, m] = sum_k b[k, n] * xT_shard[k, m]      (n, k local to core)

i.e. matmul(psum, lhsT=b_chunk[k,n], rhs=xT_chunk[k,m]) with the weight
chunk stationary. x is transposed on the host (layout prep, not compute) so
the contraction index k lands on SBUF partitions; the output comes back
transposed and is transposed back on the host during the gather.

Matmuls run as float32r (full-rate 1 cycle/row at N>=512, vs 4 cycles/row
for strict fp32) on f32 data via bitcast; accumulation is fp32 in PSUM.
"""

import numpy as np

import concourse.bass as bass
import concourse.mybir as mybir
import concourse.tile as tile
from concourse import bacc
from concourse.bass_utils import run_bass_kernel_spmd

N_CORES = 8
N_ROWS = 8192            # rows of x / out
D = 4096                 # layer size
BLOCK = 256              # block size
BLOCKS_PER_CORE = 2      # 16 blocks / 8 cores
K_PER_CORE = BLOCKS_PER_CORE * BLOCK   # 512 k (and n) columns per core
M_GROUP = 2048           # m columns per SBUF tile / DMA transfer (1 MiB)
N_GROUPS = N_ROWS // M_GROUP           # 4
MM_FREE = 512            # matmul moving free dim (one fp32 PSUM bank)
MT_PER_GROUP = M_GROUP // MM_FREE      # 4

USE_F32R = True

_nc_cache = None


def _build_nc():
    f32 = mybir.dt.float32
    mm_dt = mybir.dt.float32r if USE_F32R else f32

    # Bacc (not Bass): its compile() runs move_matmul_waits_to_ldweights and
    # generate_event_semaphores, which split multi-sem waits down to the 1
    # sync-wait-per-instruction the hardware supports.
    nc = bacc.Bacc("TRN2")
    xT = nc.dram_tensor("xT", [K_PER_CORE, N_ROWS], f32, kind="ExternalInput")
    bblk = nc.dram_tensor(
        "bblk", [BLOCKS_PER_CORE, BLOCK, BLOCK], f32, kind="ExternalInput"
    )
    outT = nc.dram_tensor("outT", [K_PER_CORE, N_ROWS], f32, kind="ExternalOutput")

    with tile.TileContext(nc) as tc:
        with (
            tc.tile_pool(name="bpool", bufs=1) as bpool,
            tc.tile_pool(name="xpool", bufs=4) as xpool,
            tc.tile_pool(name="xrpool", bufs=6) as xrpool,
            tc.tile_pool(name="opool", bufs=3) as opool,
            tc.tile_pool(name="pspool", bufs=8, space="PSUM") as pspool,
        ):
            # --- weights: load the 2 diagonal blocks, tanh once ---
            # column layout of b tiles: chunk (blk, kc) covers 256 cols at
            # (blk*2+kc)*256, holding b[k_chunk, n] for n in [0, 256).
            b_raw = bpool.tile([128, 1024], f32, name="b_raw")
            b_tanh = bpool.tile([128, 1024], f32, name="b_tanh")
            b_mm = bpool.tile([128, 1024], mm_dt, name="b_mm")
            # single DMA for all 4 [128, 256] weight chunks (keeps the tanh's
            # wait count at one semaphore): SBUF col chunk (blk*2+kc)*256
            # holds bblk[blk, kc*128 + p, n]
            nc.sync.dma_start(
                out=b_raw[:].rearrange("p (b kc n) -> p b kc n", b=2, kc=2),
                in_=bblk[:].rearrange("b (kc p) n -> p b kc n", p=128),
            )
            nc.scalar.activation(
                b_tanh[:], b_raw[:], mybir.ActivationFunctionType.Tanh
            )
            if USE_F32R:
                # fp32r is a distinct encoding: matmul operands must be
                # produced by an op that rounds to fp32r
                nc.vector.tensor_copy(b_mm[:], b_tanh[:])
            else:
                b_mm = b_tanh

            # --- stream xT in 1 MiB tiles: (q = k-chunk of 128, g = m group).
            # HWDGE loads into f32 landing tiles, DVE rounds to f32r.
            # Emission is interleaved with the compute loop: the Tile
            # scheduler's priority follows program order, so emitting all
            # casts up front lets them starve the PSUM evacuations on DVE at
            # group boundaries (measured as a multi-us store stall). Each
            # group's compute body emits the NEXT group's loads and casts so
            # evacs and casts alternate naturally on DVE.
            xts = {}

            def emit_load(q, g):
                t0 = xpool.tile([128, M_GROUP], f32, name=f"xl{q}_{g}", tag="xl")
                nc.sync.dma_start(
                    out=t0[:],
                    in_=xT[
                        q * 128 : (q + 1) * 128,
                        g * M_GROUP : (g + 1) * M_GROUP,
                    ],
                )
                return t0

            def emit_cast(q, g, t0):
                t = xrpool.tile([128, M_GROUP], mm_dt, name=f"xt{q}_{g}", tag="xt")
                nc.vector.tensor_copy(t[:], t0[:])
                xts[(q, g)] = t

            USE_DMA_CAST = True
            landing = {}
            for g in range(N_GROUPS):
                for q in range(4):
                    if USE_DMA_CAST:
                        # f32->f32r rounding inline in the SDMA datapath —
                        # no landing buffer or DVE cast needed
                        t = xrpool.tile(
                            [128, M_GROUP], mm_dt, name=f"xt{q}_{g}", tag="xt"
                        )
                        nc.gpsimd.dma_start(
                            out=t[:],
                            in_=xT[
                                q * 128 : (q + 1) * 128,
                                g * M_GROUP : (g + 1) * M_GROUP,
                            ],
                        )
                        xts[(q, g)] = t
                    else:
                        t0 = emit_load(q, g)
                        emit_cast(q, g, t0)

            # --- matmuls: psum[n 128, m 512] += b[k,n].T @ xT[k,m] over kc ---
            for g in range(N_GROUPS):
                sub = 0  # (blk, ncol) sub-iteration within the group
                for blk in range(BLOCKS_PER_CORE):
                    for ncol in range(2):  # n chunk of 128 within the block
                        out_sb = opool.tile([128, M_GROUP], f32, name="out_sb")
                        for mt in range(MT_PER_GROUP):
                            ps = pspool.tile([128, MM_FREE], f32, name="ps")
                            for kc in range(2):
                                q = blk * 2 + kc
                                lcol = ((blk * 2 + kc) * 2 + ncol) * 128
                                nc.tensor.matmul(
                                    ps[:],
                                    lhsT=b_mm[:, lcol : lcol + 128],
                                    rhs=xts[(q, g)][
                                        :, mt * MM_FREE : (mt + 1) * MM_FREE
                                    ],
                                    start=(kc == 0),
                                    stop=(kc == 1),
                                )
                            nc.vector.tensor_copy(
                                out_sb[:, mt * MM_FREE : (mt + 1) * MM_FREE], ps[:]
                            )
                        r0 = blk * 256 + ncol * 128
                        # stores go out on the ACT HWDGE ring so they don't
                        # queue behind the SP-ring loads
                        nc.scalar.dma_start(
                            out=outT[r0 : r0 + 128, g * M_GROUP : (g + 1) * M_GROUP],
                            in_=out_sb[:],
                        )
                        sub += 1
    nc.compile()
    return nc


def _get_nc():
    global _nc_cache
    if _nc_cache is None:
        _nc_cache = _build_nc()
    return _nc_cache


def _make_in_maps(x, blocks):
    xT = np.ascontiguousarray(x.T)  # [4096, 8192]
    in_maps = []
    for c in range(N_CORES):
        k0 = c * K_PER_CORE
        bstack = np.stack(
            [
                blocks[
                    k0 + i * BLOCK : k0 + (i + 1) * BLOCK,
                    k0 + i * BLOCK : k0 + (i + 1) * BLOCK,
                ]
                for i in range(BLOCKS_PER_CORE)
            ]
        )
        in_maps.append(
            {"xT": xT[k0 : k0 + K_PER_CORE, :], "bblk": np.ascontiguousarray(bstack)}
        )
    return in_maps


def _run(x, blocks, **spmd_kwargs):
    res = run_bass_kernel_spmd(
        _get_nc(), _make_in_maps(x, blocks), core_ids=list(range(N_CORES)),
        **spmd_kwargs,
    )
    out = np.empty((N_ROWS, D), np.float32)
    for c in range(N_CORES):
        out[:, c * K_PER_CORE : (c + 1) * K_PER_CORE] = res.results[c]["outT"].T
    return out, res


def kernel(x, blocks, mask=None):
    out, _ = _run(np.asarray(x), np.asarray(blocks))
    return out
